# revision 8
# baseline (speedup 1.0000x reference)
"""BiLSTM language-model kernel for 8 Trainium2 NeuronCores.

Reference computation (backward LSTM direction is dead code in the reference):
    x  = emb[input]                          # (B=8, T=512, E=512)
    xg = x @ W_ih_fwd.T + b_ih + b_hh        # (T, B, 4H)
    h  = LSTM-scan(xg, W_hh_fwd)             # (T, B, H)
    out = h @ W_out.T + b_out                # (B, T, V=32000)

Distribution strategy (v2):
  - Embedding lookup: host-side (pure indexed copy of inputs).
  - xg GEMM: chunk 0 computed locally on every core straight into SBUF (the
    scan can start without waiting on any collective); chunks 1-7 sharded
    over T across the 8 cores, one AllGather that completes under the
    first 64 scan steps.
  - LSTM scan: replicated on all 8 cores, 64 LDW+MM pairs per step at the
    ~27ns/pair weight-load floor.  Gate groups ordered (g, f, i, o) with
    each gate's add+activation issued right after its PSUM completes, so
    most of the nonlinearity chain hides under the matmul burst.  The
    o-gate's xg addend is folded into PSUM with an identity-stationary
    matmul so sigmoid(o) reads PSUM directly - the post-burst critical
    path is just sig_o -> tanh_c -> h.
  - Output GEMM: vocab-sharded (4000 rows/core), 2 MMs per scan step
    emitted at the head of each step so they execute inside the chain
    bubble; a dedicated 2-deep PSUM pool throttles the scheduler so it
    cannot clump them ahead of the recurrence.
"""

import os
import numpy as np
import ml_dtypes

import concourse.bass as bass
import concourse.tile as tile
from concourse import bacc, mybir
from concourse.bass_utils import run_bass_kernel_spmd

F32 = mybir.dt.float32
BF16 = mybir.dt.bfloat16
AF = mybir.ActivationFunctionType
ALU = mybir.AluOpType

N_CORES = 8
B, T, E, H, V = 8, 512, 512, 512, 32000
G = 4 * H                   # 2048 gate rows
NM = G // 128               # 16 gate M-tiles
NK = H // 128               # 4 contraction K-tiles
TC = T // N_CORES           # 64 timesteps per chunk
NCH = T // TC               # 8 chunks
VC = V // N_CORES           # 4000 vocab rows per core
VCH = 8                     # vocab chunks in output GEMM
VN = VC // VCH              # 500 vocab per chunk
NBT = (B * T) // 128        # 32 bt-tiles in the output GEMM

_T_BUILD = int(os.environ.get("BILSTM_T_BUILD", "512"))  # dev knob: scan length

# gate m-tile group order (g, f, i, o): g first so its tanh starts earliest,
# f/i next so the c chain closes inside the burst, o last (shortest tail).
_PERM = np.concatenate([np.arange(2 * H, 3 * H), np.arange(H, 2 * H),
                        np.arange(0, H), np.arange(3 * H, 4 * H)])
SG, SF, SI, SO = 0, 1, 2, 3  # group index = m-tiles 4g..4g+4

_CACHE = {}


def _wire_ntff_hook():
    """The agent image's antenv lacks axon_hooks; synthesize it so
    run_bass_kernel_spmd(trace=True) can capture NTFF profiles."""
    import sys
    import types
    try:
        from antenv.axon_hooks import get_axon_ntff_profile_hook  # noqa: F401
        return
    except ImportError:
        pass
    try:
        import antenv
        from trn_agent_boot.trn_boot import _ntff_profile_via_ctypes
        mod = types.ModuleType("antenv.axon_hooks")
        _store = [None]
        mod.set_axon_ntff_profile_hook = lambda h: _store.__setitem__(0, h)
        mod.get_axon_ntff_profile_hook = lambda: _store[0]
        sys.modules["antenv.axon_hooks"] = mod
        antenv.axon_hooks = mod
        mod.set_axon_ntff_profile_hook(
            _ntff_profile_via_ctypes("/opt/axon/libaxon_pjrt.so"))
    except Exception:
        pass


_wire_ntff_hook()


def _build():
    if "nc" in _CACHE:
        return _CACHE["nc"]
    nc = bacc.Bacc("TRN2", target_bir_lowering=False, debug=False,
                   num_devices=N_CORES)

    # ---- DRAM I/O ----
    xtl_dram = nc.dram_tensor("xtl", [E, TC * B], BF16, kind="ExternalInput")
    xts_dram = nc.dram_tensor("xts", [E, TC * B], BF16, kind="ExternalInput")
    wih_dram = nc.dram_tensor("wih", [E, G], BF16, kind="ExternalInput")
    whh_dram = nc.dram_tensor("whh", [H, G], BF16, kind="ExternalInput")
    bg_dram = nc.dram_tensor("bg", [128, NM], F32, kind="ExternalInput")
    wout_dram = nc.dram_tensor("wout", [H, VC], BF16, kind="ExternalInput")
    bout_dram = nc.dram_tensor("bout", [128, VC], F32, kind="ExternalInput")
    ident_dram = nc.dram_tensor("ident", [128, 128], BF16, kind="ExternalInput")
    out_dram = nc.dram_tensor("out", [B, T, VC], BF16, kind="ExternalOutput")

    # xg intermediate, layout [p, m, t_local, b]
    xg_mine = nc.dram_tensor("xg_mine", [128, NM, TC, B], BF16)
    xg_all = nc.dram_tensor("xg_all", [N_CORES, 128, NM, TC, B], BF16,
                            addr_space="Shared")

    n_chunks = (_T_BUILD + TC - 1) // TC

    with tile.TileContext(nc) as tc:
        with (
            tc.tile_pool(name="wbig", bufs=1) as wbig,      # weights
            tc.tile_pool(name="wsmall", bufs=1) as wsmall,
            tc.tile_pool(name="state", bufs=1) as statep,   # scan state
            tc.tile_pool(name="hs", bufs=NBT) as hsp,       # h staging (bf16)
            tc.tile_pool(name="xgc", bufs=3) as xgcp,       # xg chunk buffers
            tc.tile_pool(name="xgst", bufs=3) as xgst,      # phase-B staging
            tc.tile_pool(name="gt", bufs=2) as gtp,         # gate tiles
            tc.tile_pool(name="ovec", bufs=2) as ovec,      # out staging
            tc.tile_pool(name="psbig", bufs=4, space="PSUM") as psbig,  # xg phase + out GEMM
            tc.tile_pool(name="psga", bufs=1, space="PSUM") as ps_a,  # g
            tc.tile_pool(name="psgb", bufs=1, space="PSUM") as ps_b,  # f
            tc.tile_pool(name="psgc", bufs=1, space="PSUM") as ps_c,  # i
            tc.tile_pool(name="psgd", bufs=1, space="PSUM") as ps_d,  # o
        ):

            # ================= phase 0: weight loads =================
            # scalar queue: scan + out-GEMM consumables in consumption order.
            # sync queue: phase-A/B inputs.
            xtl = [wsmall.tile([128, TC * B], BF16, tag=f"xtl{k}",
                               name=f"xtl{k}") for k in range(NK)]
            xts = [wsmall.tile([128, TC * B], BF16, tag=f"xts{k}",
                               name=f"xts{k}") for k in range(NK)]
            wih = [wbig.tile([128, G], BF16, tag=f"wih{k}", name=f"wih{k}")
                   for k in range(NK)]
            for k in range(NK):
                nc.sync.dma_start(xtl[k][:], xtl_dram[128 * k:128 * (k + 1), :])
                nc.sync.dma_start(wih[k][:], wih_dram[128 * k:128 * (k + 1), :])
                nc.sync.dma_start(xts[k][:], xts_dram[128 * k:128 * (k + 1), :])
            bg = wsmall.tile([128, NM], F32)
            nc.scalar.dma_start(bg[:], bg_dram[:])
            whh = wsmall.tile([128, NK, G], BF16)
            nc.scalar.dma_start(whh[:], whh_dram[:].rearrange("(k p) g -> p k g", p=128))
            ident = wsmall.tile([128, 128], BF16)
            nc.scalar.dma_start(ident[:], ident_dram[:])
            bout = wsmall.tile([128, VC], F32)
            nc.scalar.dma_start(bout[:], bout_dram[:])
            wout = []
            for v in range(VCH):
                wt = wbig.tile([128, NK, VN], BF16, tag=f"wout{v}",
                               name=f"wout{v}")
                nc.scalar.dma_start(
                    wt[:],
                    wout_dram[:, VN * v:VN * (v + 1)].rearrange(
                        "(k p) v -> p k v", p=128))
                wout.append(wt)

            # ============ phase A: local xg for chunk 0 (into SBUF) ==========
            xgl = xgcp.tile([128, NM, TC, B], BF16, tag="xgc", name="xg_loc")
            for m in range(NM):
                ps = psbig.tile([128, TC * B], F32, tag="psbig", name=f"xgl_ps{m}")
                for k in range(NK):
                    nc.tensor.matmul(
                        ps[:], wih[k][:, 128 * m:128 * (m + 1)], xtl[k][:],
                        start=(k == 0), stop=(k == NK - 1))
                nc.scalar.activation(xgl[:, m, :, :].rearrange("p t b -> p (t b)"),
                                     ps[:], AF.Identity, bias=bg[:, m:m + 1])

            # ============ phase B: my T-shard of xg -> DRAM -> AllGather =====
            for m in range(NM):
                ps = psbig.tile([128, TC * B], F32, tag="psbig", name=f"xgs_ps{m}")
                for k in range(NK):
                    nc.tensor.matmul(
                        ps[:], wih[k][:, 128 * m:128 * (m + 1)], xts[k][:],
                        start=(k == 0), stop=(k == NK - 1))
                st = xgst.tile([128, TC * B], BF16, tag="xgst", name=f"xg_st{m}")
                nc.scalar.activation(st[:], ps[:], AF.Identity,
                                     bias=bg[:, m:m + 1])
                nc.sync.dma_start(
                    xg_mine[:, m, :, :],
                    st[:].rearrange("p (t b) -> p t b", b=B))

            nc.gpsimd.collective_compute(
                "AllGather", ALU.bypass,
                ins=[xg_mine[:]], outs=[xg_all[:]],
                replica_groups=[list(range(N_CORES))])

            # ============ phase C: chunk prefetch (chunks 1..n-1) ============
            xgc = [xgl]
            for ccn in range(1, n_chunks):
                xt_c = xgcp.tile([128, NM, TC, B], BF16, tag="xgc",
                                 name=f"xgc{ccn}")
                for s8 in range(0, TC, 8):
                    nc.sync.dma_start(xt_c[:, :, s8:s8 + 8, :],
                                      xg_all[ccn][:, :, s8:s8 + 8, :])
                xgc.append(xt_c)

            # ================= scan state =================
            c_t = statep.tile([128, NK, B], F32)
            t1 = statep.tile([128, NK, B], F32)
            t2 = statep.tile([128, NK, B], F32)
            tnc = statep.tile([128, NK, B], F32)
            h0 = statep.tile([128, NK, B], BF16)
            nc.vector.memset(c_t[:], 0.0)
            nc.vector.memset(h0[:].bitcast(mybir.dt.uint16), 0)

            hs = [hsp.tile([128, NK, 128], BF16, tag="hs", name=f"hs{j}")
                  for j in range(NBT)]
            for hst in hs:
                nc.vector.memset(hst[:].bitcast(mybir.dt.uint16), 0)

            # ============ out-GEMM emission helpers ============
            gemm_ps = {}
            ot_blk = {}

            def emit_gemm_mm(j, v, k):
                if k == 0:
                    gemm_ps[(j, v)] = psbig.tile(
                        [128, VN], F32, tag="psbig", name=f"gps{j}_{v}")
                nc.tensor.matmul(
                    gemm_ps[(j, v)][:], hs[j][:, k, :],
                    wout[v][:, k, :],
                    start=(k == 0), stop=(k == NK - 1),
                    skip_group_check=True)

            def emit_gemm_out(j, v):
                ps = gemm_ps.pop((j, v))
                if v == 0:
                    ot_blk[j] = ovec.tile([128, VC], BF16, tag="ot",
                                          name=f"ot{j}")
                ot = ot_blk[j]
                nc.vector.tensor_add(ot[:, VN * v:VN * (v + 1)], ps[:],
                                     bout[:, VN * v:VN * (v + 1)])
                if v == VCH - 1:
                    dst = out_dram[:, 16 * j:16 * (j + 1), :]
                    nc.sync.dma_start(dst.rearrange("b t v -> t b v"),
                                      ot_blk.pop(j)[:])

            # ================= scan =================
            # tile_wait_until pins each step's ops into sim-time sub-slots so
            # the static per-engine order matches the intended hw pipeline
            # (the cost-model sim has ~free matmuls, which otherwise lets the
            # scheduler hoist out-GEMM work and reorder the ACT queue).
            SLOT = 0.012            # ms of sim-time per scan step
            SUB = 0.0015

            _FOLD = bool(int(os.environ.get("BILSTM_FOLD", "1")))
            for t in range(_T_BUILD):
                cc, tl = t // TC, t % TC
                xgv = xgc[cc]
                base = t * SLOT
                if t == 0:
                    def h_ap(k):
                        return h0[:, k, :]
                else:
                    jp, op = (t - 1) // 16, (t - 1) % 16
                    def h_ap(k, _j=jp, _o=op):
                        return hs[_j][:, k, B * _o:B * (_o + 1)]

                # -- out-GEMM fill: 2 MMs at step head (previous bt-tile) --
                jj = t // 16 - 1
                og_pairs = []
                if 0 <= jj < NBT:
                    idx = t % 16
                    with tc.tile_wait_until(max(0.0, base - 2 * SUB)):
                        for pair in (2 * idx, 2 * idx + 1):
                            v, k = divmod(pair, NK)
                            emit_gemm_mm(jj, v, k)
                            if k == NK - 1:
                                og_pairs.append((jj, v))

                # -- 64 LDW+MM pairs, groups (g, f, i, o), k-inner --
                ps_g = ps_a.tile([128, 4, B], F32, tag="psG", name=f"psG_{t}")
                ps_f = ps_b.tile([128, 4, B], F32, tag="psF", name=f"psF_{t}")
                ps_i = ps_c.tile([128, 4, B], F32, tag="psI", name=f"psI_{t}")
                ps_o = ps_d.tile([128, 4, B], F32, tag="psO", name=f"psO_{t}")
                group_ps = [ps_g, ps_f, ps_i, ps_o]

                sg_t = gtp.tile([128, NK, B], F32, tag="sgT", name=f"sgT_{t}")
                sf_t = gtp.tile([128, NK, B], F32, tag="sfT", name=f"sfT_{t}")
                si_t = gtp.tile([128, NK, B], F32, tag="siT", name=f"siT_{t}")
                so_t = gtp.tile([128, NK, B], F32, tag="soT", name=f"soT_{t}")

                for grp in range(4):
                    psx = group_ps[grp]
                    fold = grp == SO and _FOLD
                    with tc.tile_wait_until(base + 1 * SUB):
                        for mi in range(4):
                            m = 4 * grp + mi
                            if fold:
                                # xg_o folded into PSUM as the accumulation
                                # group's start (identity stationary)
                                nc.tensor.matmul(
                                    psx[:, mi, :], ident[:],
                                    xgv[:, m, tl, :],
                                    start=True, stop=False)
                            for k in range(NK):
                                nc.tensor.matmul(
                                    psx[:, mi, :],
                                    whh[:, k, 128 * m:128 * (m + 1)],
                                    h_ap(k),
                                    start=(k == 0 and not fold),
                                    stop=(k == NK - 1))
                    # gate nonlinearity right after the group's psum closes
                    with tc.tile_wait_until(base + (2 + grp) * SUB):
                        if grp == SG:
                            nc.vector.tensor_add(sg_t[:], psx[:],
                                                 xgv[:, 0:4, tl, :])
                            nc.scalar.activation(sg_t[:], sg_t[:], AF.Tanh)
                        elif grp == SF:
                            nc.vector.tensor_add(sf_t[:], psx[:],
                                                 xgv[:, 4:8, tl, :])
                            nc.scalar.activation(sf_t[:], sf_t[:], AF.Sigmoid)
                            nc.vector.tensor_mul(t2[:], sf_t[:], c_t[:])
                        elif grp == SI:
                            nc.vector.tensor_add(si_t[:], psx[:],
                                                 xgv[:, 8:12, tl, :])
                            nc.scalar.activation(si_t[:], si_t[:], AF.Sigmoid)
                            nc.vector.tensor_mul(t1[:], si_t[:], sg_t[:])
                        else:
                            nc.vector.tensor_add(c_t[:], t1[:], t2[:])
                            if _FOLD:
                                nc.scalar.activation(so_t[:], psx[:],
                                                     AF.Sigmoid)
                            else:
                                nc.vector.tensor_add(so_t[:], psx[:],
                                                     xgv[:, 12:16, tl, :])
                                nc.scalar.activation(so_t[:], so_t[:],
                                                     AF.Sigmoid)
                            nc.scalar.activation(tnc[:], c_t[:], AF.Tanh)
                            j, o = t // 16, t % 16
                            nc.vector.tensor_mul(
                                hs[j][:, :, B * o:B * (o + 1)],
                                so_t[:], tnc[:])

                # out-GEMM epilogue (DVE add + DMA) after the scan chain
                with tc.tile_wait_until(base + 6 * SUB):
                    for (j_, v_) in og_pairs:
                        emit_gemm_out(j_, v_)

            # tail: last bt-tile (and any skipped when _T_BUILD < T)
            done_j = max(0, _T_BUILD // 16 - 1)
            with tc.tile_wait_until(_T_BUILD * SLOT):
                for j in range(done_j, NBT):
                    for v in range(VCH):
                        for k in range(NK):
                            emit_gemm_mm(j, v, k)
                        emit_gemm_out(j, v)

    nc.compile()
    _CACHE["nc"] = nc
    return nc


def kernel(**inputs) -> np.ndarray:
    inp = np.asarray(inputs["input"])
    emb = np.asarray(inputs["emb"], dtype=np.float32)
    W_ih = np.asarray(inputs["W_ih_fwd"], dtype=np.float32)
    b_ih = np.asarray(inputs["b_ih_fwd"], dtype=np.float32)
    W_hh = np.asarray(inputs["W_hh_fwd"], dtype=np.float32)
    b_hh = np.asarray(inputs["b_hh_fwd"], dtype=np.float32)
    W_out = np.asarray(inputs["W_out"], dtype=np.float32)
    b_out = np.asarray(inputs["b_out"], dtype=np.float32)

    nc = _build()

    # host-side input prep
    x = emb[inp]                                   # (B, T, E)
    bf = ml_dtypes.bfloat16
    wihT = np.ascontiguousarray(W_ih[_PERM].T).astype(bf)   # (E, G)
    whhT = np.ascontiguousarray(W_hh[_PERM].T).astype(bf)
    bgv = (b_ih + b_hh)[_PERM].reshape(NM, 128).T.copy()    # (128, NM)
    identm = np.eye(128, dtype=bf)

    def xt_chunk(c):
        xc = x[:, TC * c:TC * (c + 1), :]          # (B, TC, E)
        return np.ascontiguousarray(
            xc.transpose(2, 1, 0).reshape(E, TC * B)).astype(bf)

    xt0 = xt_chunk(0)
    in_maps = []
    for c in range(N_CORES):
        wo = np.ascontiguousarray(W_out[VC * c:VC * (c + 1)].T).astype(bf)
        bo = np.tile(b_out[VC * c:VC * (c + 1)][None, :], (128, 1))
        in_maps.append({
            "xtl": xt0, "xts": xt_chunk(c), "wih": wihT, "whh": whhT,
            "bg": bgv, "wout": wo, "bout": np.ascontiguousarray(bo),
            "ident": identm,
        })

    res = run_bass_kernel_spmd(
        nc, in_maps, core_ids=list(range(N_CORES)),
        trace=bool(int(os.environ.get("BILSTM_TRACE", "0"))))
    _CACHE["last_res"] = res
    out = np.concatenate([res.results[c]["out"] for c in range(N_CORES)], axis=2)
    return out.astype(np.float32)


# revision 9
# speedup vs baseline: 1.0641x; 1.0641x over previous
"""BiLSTM language-model kernel for 8 Trainium2 NeuronCores.

Reference computation (backward LSTM direction is dead code in the reference):
    x  = emb[input]                          # (B=8, T=512, E=512)
    xg = x @ W_ih_fwd.T + b_ih + b_hh        # (T, B, 4H)
    h  = LSTM-scan(xg, W_hh_fwd)             # (T, B, H)
    out = h @ W_out.T + b_out                # (B, T, V=32000)

Distribution strategy (v2):
  - Embedding lookup: host-side (pure indexed copy of inputs).
  - xg GEMM: chunk 0 computed locally on every core straight into SBUF (the
    scan can start without waiting on any collective); chunks 1-7 sharded
    over T across the 8 cores, one AllGather that completes under the
    first 64 scan steps.
  - LSTM scan: replicated on all 8 cores, 64 LDW+MM pairs per step at the
    ~27ns/pair weight-load floor.  Gate groups ordered (g, f, i, o) with
    each gate's add+activation issued right after its PSUM completes, so
    most of the nonlinearity chain hides under the matmul burst.  The
    o-gate's xg addend is folded into PSUM with an identity-stationary
    matmul so sigmoid(o) reads PSUM directly - the post-burst critical
    path is just sig_o -> tanh_c -> h.
  - Output GEMM: vocab-sharded (4000 rows/core), 2 MMs per scan step
    emitted at the head of each step so they execute inside the chain
    bubble; a dedicated 2-deep PSUM pool throttles the scheduler so it
    cannot clump them ahead of the recurrence.
"""

import os
import numpy as np
import ml_dtypes

import concourse.bass as bass
import concourse.tile as tile
from concourse import bacc, mybir
from concourse.bass_utils import run_bass_kernel_spmd

F32 = mybir.dt.float32
BF16 = mybir.dt.bfloat16
AF = mybir.ActivationFunctionType
ALU = mybir.AluOpType

N_CORES = 8
B, T, E, H, V = 8, 512, 512, 512, 32000
G = 4 * H                   # 2048 gate rows
NM = G // 128               # 16 gate M-tiles
NK = H // 128               # 4 contraction K-tiles
TC = T // N_CORES           # 64 timesteps per chunk
NCH = T // TC               # 8 chunks
VC = V // N_CORES           # 4000 vocab rows per core
VCH = 8                     # vocab chunks in output GEMM
VN = VC // VCH              # 500 vocab per chunk
NBT = (B * T) // 128        # 32 bt-tiles in the output GEMM

_T_BUILD = int(os.environ.get("BILSTM_T_BUILD", "512"))  # dev knob: scan length

# gate m-tile group order (g, f, i, o): g first so its tanh starts earliest,
# f/i next so the c chain closes inside the burst, o last (shortest tail).
_PERM = np.concatenate([np.arange(2 * H, 3 * H), np.arange(H, 2 * H),
                        np.arange(0, H), np.arange(3 * H, 4 * H)])
SG, SF, SI, SO = 0, 1, 2, 3  # group index = m-tiles 4g..4g+4

_CACHE = {}


def _wire_ntff_hook():
    """The agent image's antenv lacks axon_hooks; synthesize it so
    run_bass_kernel_spmd(trace=True) can capture NTFF profiles."""
    import sys
    import types
    try:
        from antenv.axon_hooks import get_axon_ntff_profile_hook  # noqa: F401
        return
    except ImportError:
        pass
    try:
        import antenv
        from trn_agent_boot.trn_boot import _ntff_profile_via_ctypes
        mod = types.ModuleType("antenv.axon_hooks")
        _store = [None]
        mod.set_axon_ntff_profile_hook = lambda h: _store.__setitem__(0, h)
        mod.get_axon_ntff_profile_hook = lambda: _store[0]
        sys.modules["antenv.axon_hooks"] = mod
        antenv.axon_hooks = mod
        mod.set_axon_ntff_profile_hook(
            _ntff_profile_via_ctypes("/opt/axon/libaxon_pjrt.so"))
    except Exception:
        pass


_wire_ntff_hook()


def _build():
    if "nc" in _CACHE:
        return _CACHE["nc"]
    nc = bacc.Bacc("TRN2", target_bir_lowering=False, debug=False,
                   num_devices=N_CORES)

    # ---- DRAM I/O ----
    xtl_dram = nc.dram_tensor("xtl", [E, TC * B], BF16, kind="ExternalInput")
    xts_dram = nc.dram_tensor("xts", [E, TC * B], BF16, kind="ExternalInput")
    wih_dram = nc.dram_tensor("wih", [E, G], BF16, kind="ExternalInput")
    whh_dram = nc.dram_tensor("whh", [H, G], BF16, kind="ExternalInput")
    bg_dram = nc.dram_tensor("bg", [128, NM], F32, kind="ExternalInput")
    wout_dram = nc.dram_tensor("wout", [H, VC], BF16, kind="ExternalInput")
    bout_dram = nc.dram_tensor("bout", [128, VC], F32, kind="ExternalInput")
    ident_dram = nc.dram_tensor("ident", [128, 128], BF16, kind="ExternalInput")
    out_dram = nc.dram_tensor("out", [B, T, VC], BF16, kind="ExternalOutput")

    # xg intermediate, layout [p, m, t_local, b]
    xg_mine = nc.dram_tensor("xg_mine", [128, NM, TC, B], BF16)
    xg_all = nc.dram_tensor("xg_all", [N_CORES, 128, NM, TC, B], BF16,
                            addr_space="Shared")

    n_chunks = (_T_BUILD + TC - 1) // TC

    with tile.TileContext(nc) as tc:
        with (
            tc.tile_pool(name="wbig", bufs=1) as wbig,      # weights
            tc.tile_pool(name="wsmall", bufs=1) as wsmall,
            tc.tile_pool(name="state", bufs=1) as statep,   # scan state
            tc.tile_pool(name="hs", bufs=NBT) as hsp,       # h staging (bf16)
            tc.tile_pool(name="xgc", bufs=3) as xgcp,       # xg chunk buffers
            tc.tile_pool(name="xgst", bufs=3) as xgst,      # phase-B staging
            tc.tile_pool(name="gt", bufs=2) as gtp,         # gate tiles
            tc.tile_pool(name="ovec", bufs=2) as ovec,      # out staging
            tc.tile_pool(name="psbig", bufs=4, space="PSUM") as psbig,  # xg phase + out GEMM
            tc.tile_pool(name="psga", bufs=1, space="PSUM") as ps_a,  # g
            tc.tile_pool(name="psgb", bufs=1, space="PSUM") as ps_b,  # f
            tc.tile_pool(name="psgc", bufs=1, space="PSUM") as ps_c,  # i
            tc.tile_pool(name="psgd", bufs=1, space="PSUM") as ps_d,  # o
        ):

            # ================= phase 0: weight loads =================
            # scalar queue: scan + out-GEMM consumables in consumption order.
            # sync queue: phase-A/B inputs.
            xtl = [wsmall.tile([128, TC * B], BF16, tag=f"xtl{k}",
                               name=f"xtl{k}") for k in range(NK)]
            xts = [wsmall.tile([128, TC * B], BF16, tag=f"xts{k}",
                               name=f"xts{k}") for k in range(NK)]
            wih = [wbig.tile([128, G], BF16, tag=f"wih{k}", name=f"wih{k}")
                   for k in range(NK)]
            for k in range(NK):
                nc.sync.dma_start(xtl[k][:], xtl_dram[128 * k:128 * (k + 1), :])
                nc.sync.dma_start(wih[k][:], wih_dram[128 * k:128 * (k + 1), :])
                nc.sync.dma_start(xts[k][:], xts_dram[128 * k:128 * (k + 1), :])
            bg = wsmall.tile([128, NM], F32)
            nc.scalar.dma_start(bg[:], bg_dram[:])
            whh = wsmall.tile([128, NK, G], BF16)
            nc.scalar.dma_start(whh[:], whh_dram[:].rearrange("(k p) g -> p k g", p=128))
            ident = wsmall.tile([128, 128], BF16)
            nc.scalar.dma_start(ident[:], ident_dram[:])
            bout = wsmall.tile([128, VC], F32)
            nc.scalar.dma_start(bout[:], bout_dram[:])
            wout = []
            for v in range(VCH):
                wt = wbig.tile([128, NK, VN], BF16, tag=f"wout{v}",
                               name=f"wout{v}")
                nc.scalar.dma_start(
                    wt[:],
                    wout_dram[:, VN * v:VN * (v + 1)].rearrange(
                        "(k p) v -> p k v", p=128))
                wout.append(wt)

            # ============ phase A: local xg for chunk 0 (into SBUF) ==========
            xgl = xgcp.tile([128, NM, TC, B], BF16, tag="xgc", name="xg_loc")
            for m in range(NM):
                ps = psbig.tile([128, TC * B], F32, tag="psbig", name=f"xgl_ps{m}")
                for k in range(NK):
                    nc.tensor.matmul(
                        ps[:], wih[k][:, 128 * m:128 * (m + 1)], xtl[k][:],
                        start=(k == 0), stop=(k == NK - 1))
                nc.scalar.activation(xgl[:, m, :, :].rearrange("p t b -> p (t b)"),
                                     ps[:], AF.Identity, bias=bg[:, m:m + 1])

            # ============ phase B: my T-shard of xg -> DRAM -> AllGather =====
            for m in range(NM):
                ps = psbig.tile([128, TC * B], F32, tag="psbig", name=f"xgs_ps{m}")
                for k in range(NK):
                    nc.tensor.matmul(
                        ps[:], wih[k][:, 128 * m:128 * (m + 1)], xts[k][:],
                        start=(k == 0), stop=(k == NK - 1))
                st = xgst.tile([128, TC * B], BF16, tag="xgst", name=f"xg_st{m}")
                nc.scalar.activation(st[:], ps[:], AF.Identity,
                                     bias=bg[:, m:m + 1])
                nc.sync.dma_start(
                    xg_mine[:, m, :, :],
                    st[:].rearrange("p (t b) -> p t b", b=B))

            nc.gpsimd.collective_compute(
                "AllGather", ALU.bypass,
                ins=[xg_mine[:]], outs=[xg_all[:]],
                replica_groups=[list(range(N_CORES))])

            # ============ phase C: chunk prefetch (chunks 1..n-1) ============
            xgc = [xgl]
            for ccn in range(1, n_chunks):
                xt_c = xgcp.tile([128, NM, TC, B], BF16, tag="xgc",
                                 name=f"xgc{ccn}")
                for s8 in range(0, TC, 8):
                    nc.sync.dma_start(xt_c[:, :, s8:s8 + 8, :],
                                      xg_all[ccn][:, :, s8:s8 + 8, :])
                xgc.append(xt_c)

            # ================= scan state =================
            c_t = statep.tile([128, NK, B], F32)
            t1 = statep.tile([128, NK, B], F32)
            t2 = statep.tile([128, NK, B], F32)
            tnc = statep.tile([128, NK, B], F32)
            h0 = statep.tile([128, NK, B], BF16)
            nc.vector.memset(c_t[:], 0.0)
            nc.vector.memset(h0[:].bitcast(mybir.dt.uint16), 0)

            hs = [hsp.tile([128, NK, 128], BF16, tag="hs", name=f"hs{j}")
                  for j in range(NBT)]
            for hst in hs:
                nc.vector.memset(hst[:].bitcast(mybir.dt.uint16), 0)

            # ============ out-GEMM emission helpers ============
            gemm_ps = {}
            ot_blk = {}

            def emit_gemm_mm(j, v, k):
                if k == 0:
                    gemm_ps[(j, v)] = psbig.tile(
                        [128, VN], F32, tag="psbig", name=f"gps{j}_{v}")
                nc.tensor.matmul(
                    gemm_ps[(j, v)][:], hs[j][:, k, :],
                    wout[v][:, k, :],
                    start=(k == 0), stop=(k == NK - 1),
                    skip_group_check=True)

            def emit_gemm_out(j, v):
                ps = gemm_ps.pop((j, v))
                if v == 0:
                    ot_blk[j] = ovec.tile([128, VC], BF16, tag="ot",
                                          name=f"ot{j}")
                ot = ot_blk[j]
                nc.vector.tensor_add(ot[:, VN * v:VN * (v + 1)], ps[:],
                                     bout[:, VN * v:VN * (v + 1)])
                if v == VCH - 1:
                    dst = out_dram[:, 16 * j:16 * (j + 1), :]
                    nc.sync.dma_start(dst.rearrange("b t v -> t b v"),
                                      ot_blk.pop(j)[:])

            # ================= scan =================
            # tile_wait_until pins each step's ops into sim-time sub-slots so
            # the static per-engine order matches the intended hw pipeline
            # (the cost-model sim has ~free matmuls, which otherwise lets the
            # scheduler hoist out-GEMM work and reorder the ACT queue).
            SLOT = 0.012            # ms of sim-time per scan step
            SUB = 0.0015

            _FOLD = bool(int(os.environ.get("BILSTM_FOLD", "1")))
            og_queue = [(j, v, k) for j in range(NBT)
                        for v in range(VCH) for k in range(NK)]
            for t in range(_T_BUILD):
                cc, tl = t // TC, t % TC
                xgv = xgc[cc]
                base = t * SLOT
                if t == 0:
                    def h_ap(k):
                        return h0[:, k, :]
                else:
                    jp, op = (t - 1) // 16, (t - 1) % 16
                    def h_ap(k, _j=jp, _o=op):
                        return hs[_j][:, k, B * _o:B * (_o + 1)]

                # -- out-GEMM fill: 2 MMs at step head, 2 steps behind the
                # producing window so boundary MMs never wait on a fresh hs --
                og_pairs = []
                with tc.tile_wait_until(max(0.0, base - 2 * SUB)):
                    for _ in range(2):
                        if og_queue and og_queue[0][0] * 16 + 18 <= t:
                            j_, v, k = og_queue.pop(0)
                            emit_gemm_mm(j_, v, k)
                            if k == NK - 1:
                                og_pairs.append((j_, v))

                # -- 64 LDW+MM pairs, groups (g, f, i, o), k-inner --
                ps_g = ps_a.tile([128, 4, B], F32, tag="psG", name=f"psG_{t}")
                ps_f = ps_b.tile([128, 4, B], F32, tag="psF", name=f"psF_{t}")
                ps_i = ps_c.tile([128, 4, B], F32, tag="psI", name=f"psI_{t}")
                ps_o = ps_d.tile([128, 4, B], F32, tag="psO", name=f"psO_{t}")
                group_ps = [ps_g, ps_f, ps_i, ps_o]

                sg_t = gtp.tile([128, NK, B], F32, tag="sgT", name=f"sgT_{t}")
                sf_t = gtp.tile([128, NK, B], F32, tag="sfT", name=f"sfT_{t}")
                si_t = gtp.tile([128, NK, B], F32, tag="siT", name=f"siT_{t}")
                so_t = gtp.tile([128, NK, B], F32, tag="soT", name=f"soT_{t}")

                for grp in range(4):
                    psx = group_ps[grp]
                    fold = grp in (SI, SO) and _FOLD
                    with tc.tile_wait_until(base + 1 * SUB):
                        for mi in range(4):
                            m = 4 * grp + mi
                            if fold:
                                # xg_o folded into PSUM as the accumulation
                                # group's start (identity stationary)
                                nc.tensor.matmul(
                                    psx[:, mi, :], ident[:],
                                    xgv[:, m, tl, :],
                                    start=True, stop=False)
                            for k in range(NK):
                                nc.tensor.matmul(
                                    psx[:, mi, :],
                                    whh[:, k, 128 * m:128 * (m + 1)],
                                    h_ap(k),
                                    start=(k == 0 and not fold),
                                    stop=(k == NK - 1))
                    # gate nonlinearity right after the group's psum closes
                    with tc.tile_wait_until(base + (2 + grp) * SUB):
                        if grp == SG:
                            nc.vector.tensor_add(sg_t[:], psx[:],
                                                 xgv[:, 0:4, tl, :])
                            nc.scalar.activation(sg_t[:], sg_t[:], AF.Tanh)
                        elif grp == SF:
                            nc.vector.tensor_add(sf_t[:], psx[:],
                                                 xgv[:, 4:8, tl, :])
                            nc.scalar.activation(sf_t[:], sf_t[:], AF.Sigmoid)
                            nc.vector.tensor_mul(t2[:], sf_t[:], c_t[:])
                        elif grp == SI:
                            if _FOLD:
                                nc.scalar.activation(si_t[:], psx[:],
                                                     AF.Sigmoid)
                            else:
                                nc.vector.tensor_add(si_t[:], psx[:],
                                                     xgv[:, 8:12, tl, :])
                                nc.scalar.activation(si_t[:], si_t[:],
                                                     AF.Sigmoid)
                            nc.vector.tensor_mul(t1[:], si_t[:], sg_t[:])
                        else:
                            nc.vector.tensor_add(c_t[:], t1[:], t2[:])
                            if _FOLD:
                                nc.scalar.activation(so_t[:], psx[:],
                                                     AF.Sigmoid)
                            else:
                                nc.vector.tensor_add(so_t[:], psx[:],
                                                     xgv[:, 12:16, tl, :])
                                nc.scalar.activation(so_t[:], so_t[:],
                                                     AF.Sigmoid)
                            nc.scalar.activation(tnc[:], c_t[:], AF.Tanh)
                            j, o = t // 16, t % 16
                            nc.vector.tensor_mul(
                                hs[j][:, :, B * o:B * (o + 1)],
                                so_t[:], tnc[:])

                # out-GEMM epilogue (DVE add + DMA) after the scan chain
                with tc.tile_wait_until(base + 6 * SUB):
                    for (j_, v_) in og_pairs:
                        emit_gemm_out(j_, v_)

            # tail: drain the remaining out-GEMM queue
            with tc.tile_wait_until(_T_BUILD * SLOT):
                while og_queue:
                    j_, v, k = og_queue.pop(0)
                    emit_gemm_mm(j_, v, k)
                    if k == NK - 1:
                        emit_gemm_out(j_, v)

    nc.compile()
    _CACHE["nc"] = nc
    return nc


def kernel(**inputs) -> np.ndarray:
    inp = np.asarray(inputs["input"])
    emb = np.asarray(inputs["emb"], dtype=np.float32)
    W_ih = np.asarray(inputs["W_ih_fwd"], dtype=np.float32)
    b_ih = np.asarray(inputs["b_ih_fwd"], dtype=np.float32)
    W_hh = np.asarray(inputs["W_hh_fwd"], dtype=np.float32)
    b_hh = np.asarray(inputs["b_hh_fwd"], dtype=np.float32)
    W_out = np.asarray(inputs["W_out"], dtype=np.float32)
    b_out = np.asarray(inputs["b_out"], dtype=np.float32)

    nc = _build()

    # host-side input prep
    x = emb[inp]                                   # (B, T, E)
    bf = ml_dtypes.bfloat16
    wihT = np.ascontiguousarray(W_ih[_PERM].T).astype(bf)   # (E, G)
    whhT = np.ascontiguousarray(W_hh[_PERM].T).astype(bf)
    bgv = (b_ih + b_hh)[_PERM].reshape(NM, 128).T.copy()    # (128, NM)
    identm = np.eye(128, dtype=bf)

    def xt_chunk(c):
        xc = x[:, TC * c:TC * (c + 1), :]          # (B, TC, E)
        return np.ascontiguousarray(
            xc.transpose(2, 1, 0).reshape(E, TC * B)).astype(bf)

    xt0 = xt_chunk(0)
    in_maps = []
    for c in range(N_CORES):
        wo = np.ascontiguousarray(W_out[VC * c:VC * (c + 1)].T).astype(bf)
        bo = np.tile(b_out[VC * c:VC * (c + 1)][None, :], (128, 1))
        in_maps.append({
            "xtl": xt0, "xts": xt_chunk(c), "wih": wihT, "whh": whhT,
            "bg": bgv, "wout": wo, "bout": np.ascontiguousarray(bo),
            "ident": identm,
        })

    res = run_bass_kernel_spmd(
        nc, in_maps, core_ids=list(range(N_CORES)),
        trace=bool(int(os.environ.get("BILSTM_TRACE", "0"))))
    _CACHE["last_res"] = res
    out = np.concatenate([res.results[c]["out"] for c in range(N_CORES)], axis=2)
    return out.astype(np.float32)


# revision 10
# speedup vs baseline: 1.0678x; 1.0035x over previous
"""BiLSTM language-model kernel for 8 Trainium2 NeuronCores.

Reference computation (backward LSTM direction is dead code in the reference):
    x  = emb[input]                          # (B=8, T=512, E=512)
    xg = x @ W_ih_fwd.T + b_ih + b_hh        # (T, B, 4H)
    h  = LSTM-scan(xg, W_hh_fwd)             # (T, B, H)
    out = h @ W_out.T + b_out                # (B, T, V=32000)

Distribution strategy (v2):
  - Embedding lookup: host-side (pure indexed copy of inputs).
  - xg GEMM: chunk 0 computed locally on every core straight into SBUF (the
    scan can start without waiting on any collective); chunks 1-7 sharded
    over T across the 8 cores, one AllGather that completes under the
    first 64 scan steps.
  - LSTM scan: replicated on all 8 cores, 64 LDW+MM pairs per step at the
    ~27ns/pair weight-load floor.  Gate groups ordered (g, f, i, o) with
    each gate's add+activation issued right after its PSUM completes, so
    most of the nonlinearity chain hides under the matmul burst.  The
    o-gate's xg addend is folded into PSUM with an identity-stationary
    matmul so sigmoid(o) reads PSUM directly - the post-burst critical
    path is just sig_o -> tanh_c -> h.
  - Output GEMM: vocab-sharded (4000 rows/core), 2 MMs per scan step
    emitted at the head of each step so they execute inside the chain
    bubble; a dedicated 2-deep PSUM pool throttles the scheduler so it
    cannot clump them ahead of the recurrence.
"""

import os
import numpy as np
import ml_dtypes

import concourse.bass as bass
import concourse.tile as tile
from concourse import bacc, mybir
from concourse.bass_utils import run_bass_kernel_spmd

F32 = mybir.dt.float32
BF16 = mybir.dt.bfloat16
AF = mybir.ActivationFunctionType
ALU = mybir.AluOpType

N_CORES = 8
B, T, E, H, V = 8, 512, 512, 512, 32000
G = 4 * H                   # 2048 gate rows
NM = G // 128               # 16 gate M-tiles
NK = H // 128               # 4 contraction K-tiles
TC = T // N_CORES           # 64 timesteps per chunk
NCH = T // TC               # 8 chunks
VC = V // N_CORES           # 4000 vocab rows per core
VCH = 8                     # vocab chunks in output GEMM
VN = VC // VCH              # 500 vocab per chunk
NBT = (B * T) // 128        # 32 bt-tiles in the output GEMM

_T_BUILD = int(os.environ.get("BILSTM_T_BUILD", "512"))  # dev knob: scan length

# gate m-tile group order (g, f, i, o): g first so its tanh starts earliest,
# f/i next so the c chain closes inside the burst, o last (shortest tail).
_PERM = np.concatenate([np.arange(2 * H, 3 * H), np.arange(H, 2 * H),
                        np.arange(0, H), np.arange(3 * H, 4 * H)])
SG, SF, SI, SO = 0, 1, 2, 3  # group index = m-tiles 4g..4g+4

_CACHE = {}


def _wire_ntff_hook():
    """The agent image's antenv lacks axon_hooks; synthesize it so
    run_bass_kernel_spmd(trace=True) can capture NTFF profiles."""
    import sys
    import types
    try:
        from antenv.axon_hooks import get_axon_ntff_profile_hook  # noqa: F401
        return
    except ImportError:
        pass
    try:
        import antenv
        from trn_agent_boot.trn_boot import _ntff_profile_via_ctypes
        mod = types.ModuleType("antenv.axon_hooks")
        _store = [None]
        mod.set_axon_ntff_profile_hook = lambda h: _store.__setitem__(0, h)
        mod.get_axon_ntff_profile_hook = lambda: _store[0]
        sys.modules["antenv.axon_hooks"] = mod
        antenv.axon_hooks = mod
        mod.set_axon_ntff_profile_hook(
            _ntff_profile_via_ctypes("/opt/axon/libaxon_pjrt.so"))
    except Exception:
        pass


_wire_ntff_hook()


def _build():
    if "nc" in _CACHE:
        return _CACHE["nc"]
    nc = bacc.Bacc("TRN2", target_bir_lowering=False, debug=False,
                   num_devices=N_CORES)

    # ---- DRAM I/O ----
    xtl_dram = nc.dram_tensor("xtl", [E, TC * B], BF16, kind="ExternalInput")
    xts_dram = nc.dram_tensor("xts", [E, TC * B], BF16, kind="ExternalInput")
    wih_dram = nc.dram_tensor("wih", [E, G], BF16, kind="ExternalInput")
    whh_dram = nc.dram_tensor("whh", [H, G], BF16, kind="ExternalInput")
    bg_dram = nc.dram_tensor("bg", [128, NM], F32, kind="ExternalInput")
    wout_dram = nc.dram_tensor("wout", [H, VC], BF16, kind="ExternalInput")
    bout_dram = nc.dram_tensor("bout", [128, VC], F32, kind="ExternalInput")
    ident_dram = nc.dram_tensor("ident", [128, 128], BF16, kind="ExternalInput")
    out_dram = nc.dram_tensor("out", [B, T, VC], BF16, kind="ExternalOutput")

    # xg intermediate, layout [p, m, t_local, b]
    xg_mine = nc.dram_tensor("xg_mine", [128, NM, TC, B], BF16)
    xg_all = nc.dram_tensor("xg_all", [N_CORES, 128, NM, TC, B], BF16,
                            addr_space="Shared")

    n_chunks = (_T_BUILD + TC - 1) // TC

    with tile.TileContext(nc) as tc:
        with (
            tc.tile_pool(name="wbig", bufs=1) as wbig,      # weights
            tc.tile_pool(name="wsmall", bufs=1) as wsmall,
            tc.tile_pool(name="state", bufs=1) as statep,   # scan state
            tc.tile_pool(name="hs", bufs=NBT) as hsp,       # h staging (bf16)
            tc.tile_pool(name="xgc", bufs=3) as xgcp,       # xg chunk buffers
            tc.tile_pool(name="xgst", bufs=3) as xgst,      # phase-B staging
            tc.tile_pool(name="gt", bufs=2) as gtp,         # gate tiles
            tc.tile_pool(name="ovec", bufs=2) as ovec,      # out staging
            tc.tile_pool(name="psbig", bufs=4, space="PSUM") as psbig,  # xg phase + out GEMM
            tc.tile_pool(name="psga", bufs=1, space="PSUM") as ps_a,  # g
            tc.tile_pool(name="psgb", bufs=1, space="PSUM") as ps_b,  # f
            tc.tile_pool(name="psgc", bufs=1, space="PSUM") as ps_c,  # i
            tc.tile_pool(name="psgd", bufs=1, space="PSUM") as ps_d,  # o
        ):

            # ================= phase 0: weight loads =================
            # scalar queue: scan + out-GEMM consumables in consumption order.
            # sync queue: phase-A/B inputs.
            xtl = [wsmall.tile([128, TC * B], BF16, tag=f"xtl{k}",
                               name=f"xtl{k}") for k in range(NK)]
            xts = [wsmall.tile([128, TC * B], BF16, tag=f"xts{k}",
                               name=f"xts{k}") for k in range(NK)]
            wih = [wbig.tile([128, G], BF16, tag=f"wih{k}", name=f"wih{k}")
                   for k in range(NK)]
            for k in range(NK):
                nc.sync.dma_start(xtl[k][:], xtl_dram[128 * k:128 * (k + 1), :])
                nc.sync.dma_start(wih[k][:], wih_dram[128 * k:128 * (k + 1), :])
                nc.sync.dma_start(xts[k][:], xts_dram[128 * k:128 * (k + 1), :])
            bg = wsmall.tile([128, NM], F32)
            nc.scalar.dma_start(bg[:], bg_dram[:])
            whh = wsmall.tile([128, NK, G], BF16)
            nc.scalar.dma_start(whh[:], whh_dram[:].rearrange("(k p) g -> p k g", p=128))
            ident = wsmall.tile([128, 128], BF16)
            nc.scalar.dma_start(ident[:], ident_dram[:])
            bout = wsmall.tile([128, VC], F32)
            nc.scalar.dma_start(bout[:], bout_dram[:])
            wout = []
            for v in range(VCH):
                wt = wbig.tile([128, NK, VN], BF16, tag=f"wout{v}",
                               name=f"wout{v}")
                nc.scalar.dma_start(
                    wt[:],
                    wout_dram[:, VN * v:VN * (v + 1)].rearrange(
                        "(k p) v -> p k v", p=128))
                wout.append(wt)

            # ============ phase A: local xg for chunk 0 (into SBUF) ==========
            xgl = xgcp.tile([128, NM, TC, B], BF16, tag="xgc", name="xg_loc")
            for m in range(NM):
                ps = psbig.tile([128, TC * B], F32, tag="psbig", name=f"xgl_ps{m}")
                for k in range(NK):
                    nc.tensor.matmul(
                        ps[:], wih[k][:, 128 * m:128 * (m + 1)], xtl[k][:],
                        start=(k == 0), stop=(k == NK - 1))
                nc.scalar.activation(xgl[:, m, :, :].rearrange("p t b -> p (t b)"),
                                     ps[:], AF.Identity, bias=bg[:, m:m + 1])

            # ============ phase B: my T-shard of xg -> DRAM -> AllGather =====
            for m in range(NM):
                ps = psbig.tile([128, TC * B], F32, tag="psbig", name=f"xgs_ps{m}")
                for k in range(NK):
                    nc.tensor.matmul(
                        ps[:], wih[k][:, 128 * m:128 * (m + 1)], xts[k][:],
                        start=(k == 0), stop=(k == NK - 1))
                st = xgst.tile([128, TC * B], BF16, tag="xgst", name=f"xg_st{m}")
                nc.scalar.activation(st[:], ps[:], AF.Identity,
                                     bias=bg[:, m:m + 1])
                nc.sync.dma_start(
                    xg_mine[:, m, :, :],
                    st[:].rearrange("p (t b) -> p t b", b=B))

            nc.gpsimd.collective_compute(
                "AllGather", ALU.bypass,
                ins=[xg_mine[:]], outs=[xg_all[:]],
                replica_groups=[list(range(N_CORES))])

            # ============ phase C: chunk prefetch (chunks 1..n-1) ============
            xgc = [xgl]
            for ccn in range(1, n_chunks):
                xt_c = xgcp.tile([128, NM, TC, B], BF16, tag="xgc",
                                 name=f"xgc{ccn}")
                for s8 in range(0, TC, 8):
                    nc.sync.dma_start(xt_c[:, :, s8:s8 + 8, :],
                                      xg_all[ccn][:, :, s8:s8 + 8, :])
                xgc.append(xt_c)

            # ================= scan state =================
            c_t = statep.tile([128, NK, B], F32)
            t1 = statep.tile([128, NK, B], F32)
            t2 = statep.tile([128, NK, B], F32)
            tnc = statep.tile([128, NK, B], F32)
            h0 = statep.tile([128, NK, B], BF16)
            nc.vector.memset(c_t[:], 0.0)
            nc.vector.memset(h0[:].bitcast(mybir.dt.uint16), 0)

            hs = [hsp.tile([128, NK, 128], BF16, tag="hs", name=f"hs{j}")
                  for j in range(NBT)]
            for hst in hs:
                nc.vector.memset(hst[:].bitcast(mybir.dt.uint16), 0)

            # ============ out-GEMM emission helpers ============
            gemm_ps = {}
            ot_blk = {}

            def emit_gemm_mm(j, v, k):
                if k == 0:
                    gemm_ps[(j, v)] = psbig.tile(
                        [128, VN], F32, tag="psbig", name=f"gps{j}_{v}")
                nc.tensor.matmul(
                    gemm_ps[(j, v)][:], hs[j][:, k, :],
                    wout[v][:, k, :],
                    start=(k == 0), stop=(k == NK - 1),
                    skip_group_check=True)

            def emit_gemm_out(j, v):
                ps = gemm_ps.pop((j, v))
                if v == 0:
                    ot_blk[j] = ovec.tile([128, VC], BF16, tag="ot",
                                          name=f"ot{j}")
                ot = ot_blk[j]
                nc.vector.tensor_add(ot[:, VN * v:VN * (v + 1)], ps[:],
                                     bout[:, VN * v:VN * (v + 1)])
                if v == VCH - 1:
                    dst = out_dram[:, 16 * j:16 * (j + 1), :]
                    nc.sync.dma_start(dst.rearrange("b t v -> t b v"),
                                      ot_blk.pop(j)[:])

            # ================= scan =================
            # tile_wait_until pins each step's ops into sim-time sub-slots so
            # the static per-engine order matches the intended hw pipeline
            # (the cost-model sim has ~free matmuls, which otherwise lets the
            # scheduler hoist out-GEMM work and reorder the ACT queue).
            SLOT = 0.012            # ms of sim-time per scan step
            SUB = 0.0015

            _FOLD = bool(int(os.environ.get("BILSTM_FOLD", "1")))
            og_queue = [(j, v, k) for j in range(NBT)
                        for v in range(VCH) for k in range(NK)]
            for t in range(_T_BUILD):
                cc, tl = t // TC, t % TC
                xgv = xgc[cc]
                base = t * SLOT
                if t == 0:
                    def h_ap(k):
                        return h0[:, k, :]
                else:
                    jp, op = (t - 1) // 16, (t - 1) % 16
                    def h_ap(k, _j=jp, _o=op):
                        return hs[_j][:, k, B * _o:B * (_o + 1)]

                # -- out-GEMM fill: 2 MMs at step head, 2 steps behind the
                # producing window so boundary MMs never wait on a fresh hs --
                og_pairs = []
                backlog = len(og_queue) - (NBT * VCH * NK
                                           - max(0, (t // 16) * VCH * NK))
                nmax = 3 if backlog > 4 else 2
                with tc.tile_wait_until(max(0.0, base - 2 * SUB)):
                    for _ in range(nmax):
                        if og_queue and og_queue[0][0] * 16 + 17 <= t:
                            j_, v, k = og_queue.pop(0)
                            emit_gemm_mm(j_, v, k)
                            if k == NK - 1:
                                og_pairs.append((j_, v))

                # -- 64 LDW+MM pairs, groups (g, f, i, o), k-inner --
                ps_g = ps_a.tile([128, 4, B], F32, tag="psG", name=f"psG_{t}")
                ps_f = ps_b.tile([128, 4, B], F32, tag="psF", name=f"psF_{t}")
                ps_i = ps_c.tile([128, 4, B], F32, tag="psI", name=f"psI_{t}")
                ps_o = ps_d.tile([128, 4, B], F32, tag="psO", name=f"psO_{t}")
                group_ps = [ps_g, ps_f, ps_i, ps_o]

                sg_t = gtp.tile([128, NK, B], F32, tag="sgT", name=f"sgT_{t}")
                sf_t = gtp.tile([128, NK, B], F32, tag="sfT", name=f"sfT_{t}")
                si_t = gtp.tile([128, NK, B], F32, tag="siT", name=f"siT_{t}")
                so_t = gtp.tile([128, NK, B], F32, tag="soT", name=f"soT_{t}")

                for grp in range(4):
                    psx = group_ps[grp]
                    fold = grp in (SI, SO) and _FOLD
                    with tc.tile_wait_until(base + 1 * SUB):
                        for mi in range(4):
                            m = 4 * grp + mi
                            if fold:
                                # xg_o folded into PSUM as the accumulation
                                # group's start (identity stationary)
                                nc.tensor.matmul(
                                    psx[:, mi, :], ident[:],
                                    xgv[:, m, tl, :],
                                    start=True, stop=False)
                            for k in range(NK):
                                nc.tensor.matmul(
                                    psx[:, mi, :],
                                    whh[:, k, 128 * m:128 * (m + 1)],
                                    h_ap(k),
                                    start=(k == 0 and not fold),
                                    stop=(k == NK - 1))
                    # gate nonlinearity right after the group's psum closes
                    with tc.tile_wait_until(base + (2 + grp) * SUB):
                        if grp == SG:
                            nc.vector.tensor_add(sg_t[:], psx[:],
                                                 xgv[:, 0:4, tl, :])
                            nc.scalar.activation(sg_t[:], sg_t[:], AF.Tanh)
                        elif grp == SF:
                            nc.vector.tensor_add(sf_t[:], psx[:],
                                                 xgv[:, 4:8, tl, :])
                            nc.scalar.activation(sf_t[:], sf_t[:], AF.Sigmoid)
                            nc.vector.tensor_mul(t2[:], sf_t[:], c_t[:])
                        elif grp == SI:
                            if _FOLD:
                                nc.scalar.activation(si_t[:], psx[:],
                                                     AF.Sigmoid)
                            else:
                                nc.vector.tensor_add(si_t[:], psx[:],
                                                     xgv[:, 8:12, tl, :])
                                nc.scalar.activation(si_t[:], si_t[:],
                                                     AF.Sigmoid)
                            nc.vector.tensor_mul(t1[:], si_t[:], sg_t[:])
                        else:
                            nc.vector.tensor_add(c_t[:], t1[:], t2[:])
                            if _FOLD:
                                nc.scalar.activation(so_t[:], psx[:],
                                                     AF.Sigmoid)
                            else:
                                nc.vector.tensor_add(so_t[:], psx[:],
                                                     xgv[:, 12:16, tl, :])
                                nc.scalar.activation(so_t[:], so_t[:],
                                                     AF.Sigmoid)
                            nc.scalar.activation(tnc[:], c_t[:], AF.Tanh)
                            j, o = t // 16, t % 16
                            nc.vector.tensor_mul(
                                hs[j][:, :, B * o:B * (o + 1)],
                                so_t[:], tnc[:])

                # out-GEMM epilogue (DVE add + DMA) after the scan chain
                with tc.tile_wait_until(base + 6 * SUB):
                    for (j_, v_) in og_pairs:
                        emit_gemm_out(j_, v_)

            # tail: drain the remaining out-GEMM queue
            with tc.tile_wait_until(_T_BUILD * SLOT):
                while og_queue:
                    j_, v, k = og_queue.pop(0)
                    emit_gemm_mm(j_, v, k)
                    if k == NK - 1:
                        emit_gemm_out(j_, v)

    nc.compile()
    _CACHE["nc"] = nc
    return nc


def kernel(**inputs) -> np.ndarray:
    inp = np.asarray(inputs["input"])
    emb = np.asarray(inputs["emb"], dtype=np.float32)
    W_ih = np.asarray(inputs["W_ih_fwd"], dtype=np.float32)
    b_ih = np.asarray(inputs["b_ih_fwd"], dtype=np.float32)
    W_hh = np.asarray(inputs["W_hh_fwd"], dtype=np.float32)
    b_hh = np.asarray(inputs["b_hh_fwd"], dtype=np.float32)
    W_out = np.asarray(inputs["W_out"], dtype=np.float32)
    b_out = np.asarray(inputs["b_out"], dtype=np.float32)

    nc = _build()

    # host-side input prep
    x = emb[inp]                                   # (B, T, E)
    bf = ml_dtypes.bfloat16
    wihT = np.ascontiguousarray(W_ih[_PERM].T).astype(bf)   # (E, G)
    whhT = np.ascontiguousarray(W_hh[_PERM].T).astype(bf)
    bgv = (b_ih + b_hh)[_PERM].reshape(NM, 128).T.copy()    # (128, NM)
    identm = np.eye(128, dtype=bf)

    def xt_chunk(c):
        xc = x[:, TC * c:TC * (c + 1), :]          # (B, TC, E)
        return np.ascontiguousarray(
            xc.transpose(2, 1, 0).reshape(E, TC * B)).astype(bf)

    xt0 = xt_chunk(0)
    in_maps = []
    for c in range(N_CORES):
        wo = np.ascontiguousarray(W_out[VC * c:VC * (c + 1)].T).astype(bf)
        bo = np.tile(b_out[VC * c:VC * (c + 1)][None, :], (128, 1))
        in_maps.append({
            "xtl": xt0, "xts": xt_chunk(c), "wih": wihT, "whh": whhT,
            "bg": bgv, "wout": wo, "bout": np.ascontiguousarray(bo),
            "ident": identm,
        })

    res = run_bass_kernel_spmd(
        nc, in_maps, core_ids=list(range(N_CORES)),
        trace=bool(int(os.environ.get("BILSTM_TRACE", "0"))))
    _CACHE["last_res"] = res
    out = np.concatenate([res.results[c]["out"] for c in range(N_CORES)], axis=2)
    return out.astype(np.float32)
